# revision 16
# baseline (speedup 1.0000x reference)
"""Trainium2 Bass kernel for a causal single-head attention block.

reference:
    K = x @ Wk; Q = x @ Wq; V = x @ Wv          # x [B,T,C], W [C,H]
    scores = (Q @ K^T) * C**-0.5, causal masked
    out = softmax(scores) @ V                    # [B,T,H]

B=512, T=256, C=384, H=64. Pure data parallel over batch across 8 cores
(64 batches per core); the three projection weights are replicated.

Device-side dataflow (per pair of batches):
    x^T tiles [C(part), T] are pre-transposed on the host (x is read only
    once, so the transpose rides along with the mandatory host->device copy).
    a = [Wk|Wk]^T @ x^T  -> psum a [128, T]:  rows 64:128 = K^T
    b = [Wv|Wq]^T @ x^T  -> psum b [128, T]:  rows 0:64 = V^T, 64:128 = Q^T
    (K^T and Q^T both live at partition base 64 because walrus requires
    matmul lhsT/rhs to start at the same SB partition index.)
  per batch:
    V [t, h] via PE transpose of V^T
    scoresT = K @ Q^T    (transposed scores [s, t]; the s>t block of the
           s-tile-1 half is fully masked and never computed)
    expT = exp(scoresT * scale); causal mask on the two diagonal 128x128
           blocks with one strided gpsimd affine_select
    out_unnorm[t, 0:64], denom[t] = expT^T @ [V | ones]   (ones column folds
           the softmax denominator into the PV matmul)
    out[t, h] = out_unnorm[t, h] * (1 / denom[t])  (broadcast tensor_mul)
"""

import os
import sys

for _p in ("/opt/trn_rl_repo", "/root/.axon_site/_ro/trn_rl_repo"):
    if os.path.isdir(_p) and _p not in sys.path:
        sys.path.append(_p)

from contextlib import ExitStack

import ml_dtypes
import numpy as np

import concourse.bass as bass
import concourse.tile as tile
from concourse import bacc, mybir
from concourse.bass_utils import run_bass_kernel_spmd
from concourse.masks import make_identity

B, T, C, H = 512, 256, 384, 64
N_CORES = 8
BPC = B // N_CORES  # batches per core
SCALE = float(C) ** -0.5
NCT = C // 128  # contraction tiles for the projections

F32 = mybir.dt.float32


class Cfg:
    cdt = mybir.dt.bfloat16  # compute dtype on the PE array
    np_cdt = ml_dtypes.bfloat16
    g = 8     # batches per DMA group
    pair = 2  # batches per projection matmul (N = pair*T <= 1024 for bf16)
    chunk = 8  # batches per x-load DMA (multiple of pair)
    skip_mask = False  # timing experiment only: wrong results when True
    loop_r = 0  # if >1, wrap the whole body in a For_i repeat loop (timing)


def build_body(ctx, tc, out, xT, wab, n_b, cfg, dbg=None):
    nc = tc.nc
    cdt = cfg.cdt
    g = cfg.g
    pair = cfg.pair
    n_groups = n_b // g

    consts = ctx.enter_context(tc.tile_pool(name="consts", bufs=1))
    xpool = ctx.enter_context(tc.tile_pool(name="x", bufs=3))
    kqpool = ctx.enter_context(tc.tile_pool(name="kq", bufs=3))
    epool = ctx.enter_context(tc.tile_pool(name="exp", bufs=3))
    vpool = ctx.enter_context(tc.tile_pool(name="v", bufs=4))
    opool = ctx.enter_context(tc.tile_pool(name="o", bufs=2))
    spool = ctx.enter_context(tc.tile_pool(name="small", bufs=6))
    psum = ctx.enter_context(tc.tile_pool(name="ps", bufs=2, space="PSUM"))

    # --- constants ---------------------------------------------------------
    # wab [C, 4H] = [0|Wk|Wv|Wq]: a-lhsT = cols 0:128, b-lhsT = cols 128:256
    # (zeros so scores can use full-128-partition operands -> FWL weight loads)
    wab_sb = consts.tile([128, NCT, 4 * H], cdt)
    nc.sync.dma_start(out=wab_sb, in_=wab.rearrange("(ct c) m -> c ct m", c=128))
    ident = consts.tile([128, 128], cdt)
    make_identity(nc, ident)

    def body(iv=None):
        n_pairs = n_b // pair
        out_tiles = {}
        xg_tiles = {}

        def stage_proj(p):
            grp, pr = divmod(p, g // pair)
            if pr == 0:
                out_tiles[grp] = opool.tile(
                    [128, g, 2, H], F32, tag="osb", name=f"osb{grp}"
                )
            out_sb = out_tiles[grp]
            b0 = pr * pair
            ch = min(cfg.chunk, g)
            if pr % (ch // pair) == 0:
                xgc = xpool.tile([128, ch, NCT, T], cdt, tag="xg")
                xg_tiles[p] = xgc
                nc.sync.dma_start(
                    out=xgc,
                    in_=xT[grp * g + b0 : grp * g + b0 + ch].rearrange(
                        "b (ct c) t -> c b ct t", c=128
                    ),
                )
            xgc = xg_tiles[p - (pr % (ch // pair))]
            xg = xgc[:, (pr % (ch // pair)) * pair :, :, :]
            # a: rows 64:128 = K^T ; b: rows 0:64 = V^T, 64:128 = Q^T
            a_ps = psum.tile([128, pair, T], F32, tag="aps", bufs=2)
            b_ps = psum.tile([128, pair, T], F32, tag="bps", bufs=2)
            for ct in range(NCT):
                nc.tensor.matmul(
                    a_ps,
                    wab_sb[:, ct, 0:128],
                    xg[:, 0:pair, ct, :],
                    start=(ct == 0),
                    stop=(ct == NCT - 1),
                )
            for ct in range(NCT):
                nc.tensor.matmul(
                    b_ps,
                    wab_sb[:, ct, 128:256],
                    xg[:, 0:pair, ct, :],
                    start=(ct == 0),
                    stop=(ct == NCT - 1),
                )
            a_sb = kqpool.tile([128, pair, T], cdt, tag="asb")
            nc.scalar.copy(a_sb, a_ps)
            b_sb = kqpool.tile([128, pair, T], cdt, tag="bsb")
            nc.vector.tensor_copy(b_sb, b_ps)
            return a_sb, b_sb

        def stage_batch(p, a_sb, b_sb):
            grp, pr = divmod(p, g // pair)
            out_sb = out_tiles[grp]
            b0 = pr * pair
            o_ps = psum.tile([128, pair, 2, H + 1], F32, tag="ops", bufs=1)
            # V natural [t, h] for the whole pair via PE transpose of the full
            # [V^T; Q^T] block (cols 0:64 of the result are V; 64:128 unused)
            v_ps = psum.tile([128, pair, 2, 128], cdt, tag="vps", bufs=1)
            for j in range(pair):
                for tt in range(2):
                    nc.tensor.transpose(
                        v_ps[:, j, tt, :],
                        b_sb[:, j, tt * 128 : (tt + 1) * 128],
                        ident,
                    )
            vones_p = vpool.tile([128, pair, 2, H + 1], cdt)
            nc.vector.tensor_copy(vones_p[:, :, :, 0:H], v_ps[:, :, :, 0:H])
            nc.gpsimd.memset(vones_p[:, :, :, H : H + 1], 1.0)
            for j in range(pair):
                bb = b0 + j
                vones = vones_p[:, j, :, :]

                # transposed scores, one psum bank [128, 384]:
                # cols 0:T = s-tile 0 (all t), T:T+128 = s-tile 1 (t>=128)
                sc_ps = psum.tile([128, T + 128], F32, tag="scps", bufs=2)
                nc.tensor.matmul(
                    sc_ps[:, 0:T],
                    a_sb[:, j, 0:128],
                    b_sb[:, j, :],
                    start=True,
                    stop=True,
                )
                nc.tensor.matmul(
                    sc_ps[:, T : T + 128],
                    a_sb[:, j, 128:T],
                    b_sb[:, j, 128:T],
                    start=True,
                    stop=True,
                )

                expT = epool.tile([128, T + 128], cdt)
                nc.scalar.activation(
                    expT,
                    sc_ps,
                    mybir.ActivationFunctionType.Exp,
                    scale=SCALE,
                )
                # causal mask on both diagonal blocks (cols 0:128 and
                # 256:384) in one strided op: keep where -s + t >= 0
                blocks = expT.rearrange("p (n c) -> p n c", c=128)[:, 0::2, :]
                if cfg.skip_mask:
                    blocks = None
                else:
                    nc.gpsimd.affine_select(
                        out=blocks,
                        in_=blocks,
                        compare_op=mybir.AluOpType.is_ge,
                        fill=0.0,
                        base=0,
                        pattern=[[0, 2], [1, 128]],
                        channel_multiplier=-1,
                    )

                # PV + folded denominator: out_unnorm = expT^T @ [V | 1]
                nc.tensor.matmul(
                    o_ps[:, j, 0, :],
                    expT[:, 0:128],
                    vones[:, 0, :],
                    start=True,
                    stop=True,
                )
                nc.tensor.matmul(
                    o_ps[:, j, 1, :],
                    expT[:, 128:T],
                    vones[:, 0, :],
                    start=True,
                    stop=False,
                )
                nc.tensor.matmul(
                    o_ps[:, j, 1, :],
                    expT[:, T : T + 128],
                    vones[:, 1, :],
                    start=False,
                    stop=True,
                )

                if dbg is not None and p == 0 and bb == 0:
                    nc.vector.tensor_copy(dbg["kq"][0:64], a_sb[64:128, 0, :])
                    nc.vector.tensor_copy(dbg["kq"][64:128], b_sb[64:128, 0, :])
                    nc.vector.tensor_copy(dbg["vones"], vones)
                    nc.vector.tensor_copy(dbg["expT"], expT)

            recip = spool.tile([128, pair, 2, 1], F32)
            nc.vector.reciprocal(recip, o_ps[:, :, :, H : H + 1])
            rbc = bass.AP(
                tensor=recip.tensor,
                offset=recip.offset,
                ap=[recip.ap[0], recip.ap[1], recip.ap[2], [0, H]],
            )
            nc.vector.tensor_mul(
                out_sb[:, b0 : b0 + pair, :, :], o_ps[:, :, :, 0:H], rbc
            )
            nc.scalar.dma_start(
                out=out[grp * g + b0 : grp * g + b0 + pair].rearrange(
                    "b (tt p) h -> p b tt h", p=128
                ),
                in_=out_sb[:, b0 : b0 + pair, :, :],
            )

        # software pipeline: projections run one pair ahead of batch ops
        pend = None
        for p in range(n_pairs):
            ab = stage_proj(p)
            if pend is not None:
                stage_batch(p - 1, *pend)
            pend = ab
        stage_batch(n_pairs - 1, *pend)

    if cfg.loop_r and cfg.loop_r > 1:
        with tc.For_i(0, cfg.loop_r, 1) as iv:
            body(iv)
    else:
        body()


def build_kernel(n_b=BPC, cfg=None, debug_taps=False):
    cfg = cfg or Cfg()
    nc = bacc.Bacc("TRN2", target_bir_lowering=False, debug=False)
    xT = nc.dram_tensor("xT", [n_b, C, T], cfg.cdt, kind="ExternalInput").ap()
    wab = nc.dram_tensor("wab", [C, 4 * H], cfg.cdt, kind="ExternalInput").ap()
    out = nc.dram_tensor("out", [n_b, T, H], F32, kind="ExternalOutput").ap()
    dbg = None
    dbg_specs = {
        "kq": [128, T],
        "vones": [128, 2, H + 1],
        "expT": [128, T + 128],
    }
    dbg_dram = {}
    if debug_taps:
        dbg_dram = {
            k: nc.dram_tensor(f"dbg_{k}", s, cfg.cdt, kind="ExternalOutput").ap()
            for k, s in dbg_specs.items()
        }

    with tile.TileContext(nc) as tc, ExitStack() as ctx:
        if debug_taps:
            dbgpool = ctx.enter_context(tc.tile_pool(name="dbg", bufs=1))
            dbg = {
                k: dbgpool.tile(s, cfg.cdt, name=f"dbgsb_{k}")
                for k, s in dbg_specs.items()
            }
        build_body(ctx, tc, out, xT, wab, n_b, cfg, dbg=dbg)
        if debug_taps:
            for k in dbg_dram:
                nc.sync.dma_start(out=dbg_dram[k], in_=dbg[k])
    nc.compile()
    return nc


def prep_inputs(x, Wk, Wq, Wv, n_cores=N_CORES, cfg=None):
    """Shard over batch + host-side pre-transpose/cast of x."""
    cfg = cfg or Cfg()
    bpc = x.shape[0] // n_cores
    wab = np.concatenate([np.zeros_like(Wk), Wk, Wv, Wq], axis=1).astype(cfg.np_cdt)
    in_maps = []
    for i in range(n_cores):
        shard = x[i * bpc : (i + 1) * bpc]
        xTs = np.ascontiguousarray(shard.transpose(0, 2, 1)).astype(cfg.np_cdt)
        in_maps.append({"xT": xTs, "wab": wab})
    return in_maps


_NC_CACHE = {}


def kernel(x, Wk, Wq, Wv):
    cfg = Cfg()
    key = (x.shape[0] // N_CORES, cfg.cdt, cfg.g, cfg.pair, cfg.chunk)
    if key not in _NC_CACHE:
        _NC_CACHE[key] = build_kernel(n_b=key[0], cfg=cfg)
    nc = _NC_CACHE[key]
    in_maps = prep_inputs(x, Wk, Wq, Wv, cfg=cfg)
    res = run_bass_kernel_spmd(nc, in_maps, list(range(N_CORES)))
    return np.concatenate([r["out"] for r in res.results], axis=0)


# revision 18
# speedup vs baseline: 1.0126x; 1.0126x over previous
"""Trainium2 Bass kernel for a causal single-head attention block.

reference:
    K = x @ Wk; Q = x @ Wq; V = x @ Wv          # x [B,T,C], W [C,H]
    scores = (Q @ K^T) * C**-0.5, causal masked
    out = softmax(scores) @ V                    # [B,T,H]

B=512, T=256, C=384, H=64. Pure data parallel over batch across 8 cores
(64 batches per core); the three projection weights are replicated.

Device-side dataflow (per pair of batches):
    x^T tiles [C(part), T] are pre-transposed on the host (x is read only
    once, so the transpose rides along with the mandatory host->device copy).
    a = [Wk|Wk]^T @ x^T  -> psum a [128, T]:  rows 64:128 = K^T
    b = [Wv|Wq]^T @ x^T  -> psum b [128, T]:  rows 0:64 = V^T, 64:128 = Q^T
    (K^T and Q^T both live at partition base 64 because walrus requires
    matmul lhsT/rhs to start at the same SB partition index.)
  per batch:
    V [t, h] via PE transpose of V^T
    scoresT = K @ Q^T    (transposed scores [s, t]; the s>t block of the
           s-tile-1 half is fully masked and never computed)
    expT = exp(scoresT * scale); causal mask on the two diagonal 128x128
           blocks with one strided gpsimd affine_select
    out_unnorm[t, 0:64], denom[t] = expT^T @ [V | ones]   (ones column folds
           the softmax denominator into the PV matmul)
    out[t, h] = out_unnorm[t, h] * (1 / denom[t])  (broadcast tensor_mul)
"""

import os
import sys

for _p in ("/opt/trn_rl_repo", "/root/.axon_site/_ro/trn_rl_repo"):
    if os.path.isdir(_p) and _p not in sys.path:
        sys.path.append(_p)

from contextlib import ExitStack

import ml_dtypes
import numpy as np

import concourse.bass as bass
import concourse.tile as tile
from concourse import bacc, mybir
from concourse.bass_utils import run_bass_kernel_spmd
from concourse.masks import make_identity

B, T, C, H = 512, 256, 384, 64
N_CORES = 8
BPC = B // N_CORES  # batches per core
SCALE = float(C) ** -0.5
NCT = C // 128  # contraction tiles for the projections

F32 = mybir.dt.float32


class Cfg:
    cdt = mybir.dt.bfloat16  # compute dtype on the PE array
    np_cdt = ml_dtypes.bfloat16
    g = 8     # batches per DMA group
    pair = 2  # batches per projection matmul (N = pair*T <= 1024 for bf16)
    chunk = 8  # batches per x-load DMA (multiple of pair)
    skip_mask = False  # timing experiment only: wrong results when True
    loop_r = 0  # if >1, wrap the whole body in a For_i repeat loop (timing)


def build_body(ctx, tc, out, xT, wab, n_b, cfg, dbg=None):
    nc = tc.nc
    cdt = cfg.cdt
    g = cfg.g
    pair = cfg.pair
    n_groups = n_b // g

    consts = ctx.enter_context(tc.tile_pool(name="consts", bufs=1))
    xpool = ctx.enter_context(tc.tile_pool(name="x", bufs=3))
    kqpool = ctx.enter_context(tc.tile_pool(name="kq", bufs=3))
    epool = ctx.enter_context(tc.tile_pool(name="exp", bufs=3))
    vpool = ctx.enter_context(tc.tile_pool(name="v", bufs=4))
    opool = ctx.enter_context(tc.tile_pool(name="o", bufs=2))
    spool = ctx.enter_context(tc.tile_pool(name="small", bufs=6))
    psum = ctx.enter_context(tc.tile_pool(name="ps", bufs=2, space="PSUM"))

    # --- constants ---------------------------------------------------------
    # wab [C, 4H] = [0|Wk|Wv|Wq]: a-lhsT = cols 0:128, b-lhsT = cols 128:256
    # (zeros so scores can use full-128-partition operands -> FWL weight loads)
    wab_sb = consts.tile([128, NCT, 4 * H], cdt)
    nc.sync.dma_start(out=wab_sb, in_=wab.rearrange("(ct c) m -> c ct m", c=128))
    ident64 = consts.tile([64, 64], cdt)
    make_identity(nc, ident64)

    def body(iv=None):
        n_pairs = n_b // pair
        out_tiles = {}
        xg_tiles = {}

        def stage_proj(p):
            grp, pr = divmod(p, g // pair)
            if pr == 0:
                out_tiles[grp] = opool.tile(
                    [128, g, 2, H], F32, tag="osb", name=f"osb{grp}"
                )
            out_sb = out_tiles[grp]
            b0 = pr * pair
            ch = min(cfg.chunk, g)
            if pr % (ch // pair) == 0:
                xgc = xpool.tile([128, ch, NCT, T], cdt, tag="xg")
                xg_tiles[p] = xgc
                nc.sync.dma_start(
                    out=xgc,
                    in_=xT[grp * g + b0 : grp * g + b0 + ch].rearrange(
                        "b (ct c) t -> c b ct t", c=128
                    ),
                )
            xgc = xg_tiles[p - (pr % (ch // pair))]
            xg = xgc[:, (pr % (ch // pair)) * pair :, :, :]
            # a: rows 64:128 = K^T ; b: rows 0:64 = V^T, 64:128 = Q^T
            a_ps = psum.tile([128, pair, T], F32, tag="aps", bufs=2)
            b_ps = psum.tile([128, pair, T], F32, tag="bps", bufs=2)
            for ct in range(NCT):
                nc.tensor.matmul(
                    a_ps,
                    wab_sb[:, ct, 0:128],
                    xg[:, 0:pair, ct, :],
                    start=(ct == 0),
                    stop=(ct == NCT - 1),
                )
            for ct in range(NCT):
                nc.tensor.matmul(
                    b_ps,
                    wab_sb[:, ct, 128:256],
                    xg[:, 0:pair, ct, :],
                    start=(ct == 0),
                    stop=(ct == NCT - 1),
                )
            a_sb = kqpool.tile([128, pair, T], cdt, tag="asb")
            nc.scalar.copy(a_sb, a_ps)
            b_sb = kqpool.tile([128, pair, T], cdt, tag="bsb")
            nc.vector.tensor_copy(b_sb, b_ps)
            return a_sb, b_sb

        def stage_batch(p, a_sb, b_sb):
            grp, pr = divmod(p, g // pair)
            out_sb = out_tiles[grp]
            b0 = pr * pair
            o_ps = psum.tile([128, pair, 2, H + 1], F32, tag="ops", bufs=1)
            # V natural [t, h] for the whole pair via PE transpose of V^T
            v_ps = psum.tile([128, pair, 2, H], cdt, tag="vps", bufs=1)
            for j in range(pair):
                for tt in range(2):
                    nc.tensor.transpose(
                        v_ps[:, j, tt, :],
                        b_sb[0:64, j, tt * 128 : (tt + 1) * 128],
                        ident64,
                    )
            vones_p = vpool.tile([128, pair, 2, H + 1], cdt)
            nc.vector.tensor_copy(vones_p[:, :, :, 0:H], v_ps)
            nc.gpsimd.memset(vones_p[:, :, :, H : H + 1], 1.0)
            for j in range(pair):
                bb = b0 + j
                vones = vones_p[:, j, :, :]

                # transposed scores, one psum bank [128, 384]:
                # cols 0:T = s-tile 0 (all t), T:T+128 = s-tile 1 (t>=128)
                sc_ps = psum.tile([128, T + 128], F32, tag="scps", bufs=2)
                nc.tensor.matmul(
                    sc_ps[:, 0:T],
                    a_sb[:, j, 0:128],
                    b_sb[:, j, :],
                    start=True,
                    stop=True,
                )
                nc.tensor.matmul(
                    sc_ps[:, T : T + 128],
                    a_sb[:, j, 128:T],
                    b_sb[:, j, 128:T],
                    start=True,
                    stop=True,
                )

                expT = epool.tile([128, T + 128], cdt)
                nc.scalar.activation(
                    expT,
                    sc_ps,
                    mybir.ActivationFunctionType.Exp,
                    scale=SCALE,
                )
                # causal mask on both diagonal blocks (cols 0:128 and
                # 256:384) in one strided op: keep where -s + t >= 0
                blocks = expT.rearrange("p (n c) -> p n c", c=128)[:, 0::2, :]
                if cfg.skip_mask:
                    blocks = None
                else:
                    nc.gpsimd.affine_select(
                        out=blocks,
                        in_=blocks,
                        compare_op=mybir.AluOpType.is_ge,
                        fill=0.0,
                        base=0,
                        pattern=[[0, 2], [1, 128]],
                        channel_multiplier=-1,
                    )

                # PV + folded denominator: out_unnorm = expT^T @ [V | 1]
                nc.tensor.matmul(
                    o_ps[:, j, 0, :],
                    expT[:, 0:128],
                    vones[:, 0, :],
                    start=True,
                    stop=True,
                )
                nc.tensor.matmul(
                    o_ps[:, j, 1, :],
                    expT[:, 128:T],
                    vones[:, 0, :],
                    start=True,
                    stop=False,
                )
                nc.tensor.matmul(
                    o_ps[:, j, 1, :],
                    expT[:, T : T + 128],
                    vones[:, 1, :],
                    start=False,
                    stop=True,
                )

                if dbg is not None and p == 0 and bb == 0:
                    nc.vector.tensor_copy(dbg["kq"][0:64], a_sb[64:128, 0, :])
                    nc.vector.tensor_copy(dbg["kq"][64:128], b_sb[64:128, 0, :])
                    nc.vector.tensor_copy(dbg["vones"], vones)
                    nc.vector.tensor_copy(dbg["expT"], expT)

            recip = spool.tile([128, pair, 2, 1], F32)
            nc.vector.reciprocal(recip, o_ps[:, :, :, H : H + 1])
            rbc = bass.AP(
                tensor=recip.tensor,
                offset=recip.offset,
                ap=[recip.ap[0], recip.ap[1], recip.ap[2], [0, H]],
            )
            nc.vector.tensor_mul(
                out_sb[:, b0 : b0 + pair, :, :], o_ps[:, :, :, 0:H], rbc
            )
            if pr == (g // pair) - 1:
                nc.scalar.dma_start(
                    out=out[grp * g : (grp + 1) * g].rearrange(
                        "b (tt p) h -> p b tt h", p=128
                    ),
                    in_=out_sb,
                )

        # software pipeline: projections run one pair ahead of batch ops
        pend = None
        for p in range(n_pairs):
            ab = stage_proj(p)
            if pend is not None:
                stage_batch(p - 1, *pend)
            pend = ab
        stage_batch(n_pairs - 1, *pend)

    if cfg.loop_r and cfg.loop_r > 1:
        hints = (
            mybir.EngineType.PE,
            mybir.EngineType.DVE,
            mybir.EngineType.Activation,
            mybir.EngineType.Pool,
            mybir.EngineType.SP,
        )
        with tc.For_i(0, cfg.loop_r, 1, hint_engines=hints) as iv:
            body(iv)
    else:
        body()


def build_kernel(n_b=BPC, cfg=None, debug_taps=False):
    cfg = cfg or Cfg()
    nc = bacc.Bacc("TRN2", target_bir_lowering=False, debug=False)
    xT = nc.dram_tensor("xT", [n_b, C, T], cfg.cdt, kind="ExternalInput").ap()
    wab = nc.dram_tensor("wab", [C, 4 * H], cfg.cdt, kind="ExternalInput").ap()
    out = nc.dram_tensor("out", [n_b, T, H], F32, kind="ExternalOutput").ap()
    dbg = None
    dbg_specs = {
        "kq": [128, T],
        "vones": [128, 2, H + 1],
        "expT": [128, T + 128],
    }
    dbg_dram = {}
    if debug_taps:
        dbg_dram = {
            k: nc.dram_tensor(f"dbg_{k}", s, cfg.cdt, kind="ExternalOutput").ap()
            for k, s in dbg_specs.items()
        }

    with tile.TileContext(nc) as tc, ExitStack() as ctx:
        if debug_taps:
            dbgpool = ctx.enter_context(tc.tile_pool(name="dbg", bufs=1))
            dbg = {
                k: dbgpool.tile(s, cfg.cdt, name=f"dbgsb_{k}")
                for k, s in dbg_specs.items()
            }
        build_body(ctx, tc, out, xT, wab, n_b, cfg, dbg=dbg)
        if debug_taps:
            for k in dbg_dram:
                nc.sync.dma_start(out=dbg_dram[k], in_=dbg[k])
    nc.compile()
    return nc


def prep_inputs(x, Wk, Wq, Wv, n_cores=N_CORES, cfg=None):
    """Shard over batch + host-side pre-transpose/cast of x."""
    cfg = cfg or Cfg()
    bpc = x.shape[0] // n_cores
    wab = np.concatenate([np.zeros_like(Wk), Wk, Wv, Wq], axis=1).astype(cfg.np_cdt)
    in_maps = []
    for i in range(n_cores):
        shard = x[i * bpc : (i + 1) * bpc]
        xTs = np.ascontiguousarray(shard.transpose(0, 2, 1)).astype(cfg.np_cdt)
        in_maps.append({"xT": xTs, "wab": wab})
    return in_maps


_NC_CACHE = {}


def kernel(x, Wk, Wq, Wv):
    cfg = Cfg()
    key = (x.shape[0] // N_CORES, cfg.cdt, cfg.g, cfg.pair, cfg.chunk)
    if key not in _NC_CACHE:
        _NC_CACHE[key] = build_kernel(n_b=key[0], cfg=cfg)
    nc = _NC_CACHE[key]
    in_maps = prep_inputs(x, Wk, Wq, Wv, cfg=cfg)
    res = run_bass_kernel_spmd(nc, in_maps, list(range(N_CORES)))
    return np.concatenate([r["out"] for r in res.results], axis=0)
